# revision 18
# baseline (speedup 1.0000x reference)
"""Trainium2 Bass kernel for the dynamic segment-aggregation module.

Computation per (clip n, channel c):
  pooled[u]  = mean_{t,h,w} x[n,c,u,...]                (U=4 segments)
  z          = relu(pooled @ W1'^T + t_row)             (BN folded into W1/t)
  kern       = softmax(z @ W2^T)                        (K=3 taps)
  out[u]     = kern[0]*x[u-1] + kern[1]*x[u] + kern[2]*x[u+1]   (zero-padded)

Sharding: data-parallel over the 8 clips -> 1 clip (4 U-segments) per
NeuronCore; tiny generator weights replicated (one 64-float tensor with
BN affine and the 1/THW pooling mean folded in host-side).

The big lever vs the fp32 version: x and out travel through HBM as
bf16 (host casts both ways), halving DMA traffic to ~25.7 MB/core
(~72 us at 358 GB/s) and unlocking the DVE 2x (2-tensor) / 4x
(1-tensor) perf modes.  The blend is fused into one ScalarE tap-strip
(k2*x[1:4], FD=4704) plus four DVE ops per slab:
    out[0]   = k1*x[0]   + t[0]          (STT, FD=1568)
    p[0:2]   = k1*x[1:3] + t[1:3]        (STT, FD=3136)
    p[2]     = k1*x[3]                   (TS,  FD=1568, 4x)
    out[1:4] = k0*x[0:3] + p[0:3]        (STT, FD=4704)
Group-0 pooling rides DVE during the load phase; group-1 pooling is
split ScalarE/DVE inside the group-0 blend window.  All 8 slab loads
are queued up-front on the Sync HWDGE ring; group-0 stores defer
behind the last load so loads keep HBM priority.
"""

import numpy as np
import ml_dtypes

import concourse.bass as bass
import concourse.bacc as bacc
import concourse.tile as tile
from concourse import mybir
from concourse.bass_utils import run_bass_kernel_spmd

U = 4          # segments per clip
C = 256        # channels
T, H, W = 8, 28, 28
THW = T * H * W            # 6272
NQ = 4                     # t-quarters per channel-group
FQ = THW // NQ             # 1568
D = 8                      # MLP hidden dim (U * alpha)
K = 3                      # conv taps
EPS = 1e-5
N_CORES = 8
NCG = C // 128             # channel groups per core

# packed small-weights layout: [W1*s/THW (D*U) | W2 (K*D) | t_row (D)]
NPACK = D * U + K * D + D    # 64

FP32 = mybir.dt.float32
BF16 = mybir.dt.bfloat16

_nc_cache = None
last_results = None        # BassKernelResults of the most recent run (for test.py)


def _bcast_ap(ap, parts=128):
    """DRAM AP replicated across `parts` partitions (partition stride 0)."""
    return bass.AP(tensor=ap.tensor, offset=ap.offset, ap=[[0, parts]] + list(ap.ap))


def _bc_free(ap, n):
    """Insert a stride-0 broadcast dim of length n right after the
    partition dim of an SBUF AP."""
    dims = [list(d) for d in ap.ap]
    return bass.AP(
        tensor=ap.tensor, offset=ap.offset, ap=[dims[0]] + [[0, n]] + dims[1:]
    )


def _build_nc():
    nc = bacc.Bacc(None, target_bir_lowering=False)
    x_h = nc.declare_dram_parameter("x", [U, C, THW], BF16, isOutput=False)
    wp_h = nc.declare_dram_parameter("wpack", [NPACK], FP32, isOutput=False)
    out_h = nc.declare_dram_parameter("out", [U, C, THW], BF16, isOutput=True)

    xg = x_h[:].rearrange("u c f -> c u f")      # [C, U, THW]
    og = out_h[:].rearrange("u c f -> c u f")

    AX = mybir.AxisListType
    OP = mybir.AluOpType
    AF = mybir.ActivationFunctionType

    with tile.TileContext(nc) as tc:
        with (
            tc.tile_pool(name="xp", bufs=8) as xp,
            tc.tile_pool(name="outp", bufs=4) as outp,
            tc.tile_pool(name="tp", bufs=2) as tp,
            tc.tile_pool(name="ppp", bufs=2) as ppp,
            tc.tile_pool(name="small", bufs=1) as small,
            tc.tile_pool(name="mlp", bufs=2) as mlp,
        ):
            # one tiny DMA for every per-core-replicated constant
            wpk = small.tile([128, NPACK], FP32)
            nc.gpsimd.dma_start(out=wpk, in_=_bcast_ap(wp_h[:]))
            w1sb = wpk[:, 0:D * U].rearrange("p (d u) -> p d u", d=D)       # [128,D,U]
            w2sb = wpk[:, D * U:D * U + K * D].rearrange(
                "p (k d) -> p k d", k=K)                                    # [128,K,D]
            t_row = wpk[:, D * U + K * D:NPACK]                             # [128,D]

            def load_slab(g, q, split=False):
                c0 = g * 128
                sl = xp.tile([128, U, FQ], BF16, tag="xslab")
                if split:
                    # two half-loads so the first pools can start sooner
                    nc.sync.dma_start(
                        out=sl[:, 0:2, :],
                        in_=xg[c0:c0 + 128, 0:2, q * FQ:(q + 1) * FQ],
                    )
                    ld = nc.sync.dma_start(
                        out=sl[:, 2:4, :],
                        in_=xg[c0:c0 + 128, 2:4, q * FQ:(q + 1) * FQ],
                    )
                else:
                    ld = nc.sync.dma_start(
                        out=sl, in_=xg[c0:c0 + 128, :, q * FQ:(q + 1) * FQ]
                    )
                return sl, ld

            def pool_u(sl, q, P, u, on_act):
                """One per-u pooling pass.  ScalarE: activation accumulator
                (self-copy carrier).  DVE: tensor_reduce (accum_out on DVE
                drops to 1x anyway; reduce avoids the dummy write)."""
                if on_act:
                    nc.scalar.activation(
                        out=sl[:, u, :], in_=sl[:, u, :], func=AF.Copy,
                        accum_out=P[:, u, q:q + 1],
                    )
                else:
                    nc.vector.reduce_sum(
                        out=P[:, u, q:q + 1], in_=sl[:, u, :], axis=AX.X
                    )

            def gen_mlp(P):
                """pooled -> relu(pooled@W1'^T + t_row) -> softmax(z@W2^T)."""
                pooled = mlp.tile([128, U], FP32, tag="pooled")
                nc.vector.reduce_sum(out=pooled, in_=P, axis=AX.X)
                zw = mlp.tile([128, D, U], FP32, tag="zw")
                nc.vector.tensor_mul(
                    out=zw, in0=w1sb, in1=_bc_free(pooled[:, :], D)
                )
                z = mlp.tile([128, D], FP32, tag="z")
                nc.vector.reduce_sum(out=z, in_=zw, axis=AX.X)
                nc.vector.tensor_add(out=z, in0=z, in1=t_row)
                nc.vector.tensor_scalar_max(out=z, in0=z, scalar1=0.0)
                lw = mlp.tile([128, K, D], FP32, tag="lw")
                nc.vector.tensor_mul(
                    out=lw, in0=w2sb, in1=_bc_free(z[:, :], K)
                )
                logit = mlp.tile([128, K], FP32, tag="logit")
                nc.vector.reduce_sum(out=logit, in_=lw, axis=AX.X)
                mx = mlp.tile([128, 1], FP32, tag="mx")
                nc.vector.reduce_max(out=mx, in_=logit, axis=AX.X,
                                     negate=True)
                nc.scalar.activation(
                    out=logit, in_=logit, func=AF.Exp, bias=mx[:, 0:1]
                )
                ssum = mlp.tile([128, 1], FP32, tag="ssum")
                nc.vector.reduce_sum(out=ssum, in_=logit, axis=AX.X)
                nc.vector.reciprocal(out=ssum, in_=ssum)
                kern = mlp.tile([128, K], FP32, tag="kern")
                nc.vector.tensor_scalar_mul(out=kern, in0=logit, scalar1=ssum[:, 0:1])
                return kern

            def blend(g, q, sl, kern, b_on_act, add2_on_dve=False):
                """One slab blend.  STT is 1x on this silicon, so the MAC
                stream is TS products (4x) + in-place TT adds (2x):
                  ot      = k1*sl          (TS, FD 6272)
                  B       = k2*sl[1:4]     (TS, FD 4704; ScalarE when it
                                            has slack -- b_on_act)
                  Cc      = k0*sl[0:3]     (TS, FD 4704)
                  ot[0:3] += B             (TT, FD 4704)
                  ot[1:3] += Cc[0:2]       (TT, FD 3136)
                  ot[3]   += Cc[2]         (TT on GpSimd, FD 1568)
                Returns the store handle."""
                c0 = g * 128
                k0, k1, k2 = kern[:, 0:1], kern[:, 1:2], kern[:, 2:3]
                ot = outp.tile([128, U, FQ], BF16, tag="outslab")
                nc.vector.tensor_scalar_mul(out=ot, in0=sl, scalar1=k1)
                B = tp.tile([128, 3, FQ], BF16, tag="B")
                if b_on_act:
                    nc.scalar.activation(
                        out=B, in_=sl[:, 1:4, :], func=AF.Copy, scale=k2
                    )
                else:
                    nc.vector.tensor_scalar_mul(
                        out=B, in0=sl[:, 1:4, :], scalar1=k2
                    )
                Cc = ppp.tile([128, 3, FQ], BF16, tag="Cc")
                nc.vector.tensor_scalar_mul(
                    out=Cc, in0=sl[:, 0:3, :], scalar1=k0
                )
                # two in-place TT adds; both FD 4704 at 2x.  (No GpSimd
                # here: concurrent Q7 SBUF traffic knocks the DVE 4x port
                # mode down to 1x — observed 0.95 -> 3.85 us.)
                nc.vector.tensor_add(
                    out=ot[:, 0:3, :], in0=ot[:, 0:3, :], in1=B
                )
                nc.vector.tensor_add(
                    out=ot[:, 1:4, :], in0=ot[:, 1:4, :], in1=Cc
                )
                st = nc.gpsimd.dma_start(
                    out=og[c0:c0 + 128, :, q * FQ:(q + 1) * FQ], in_=ot
                )
                return st

            # ---- schedule ----
            from concourse.tile_rust import add_dep_helper

            # Loads laddered two-deep: with all 8 queued at once the HWDGE
            # ring interleaves them and the FIRST slab only lands at ~17us
            # (observed), stalling pooling.  Depth-2 keeps the pipe full
            # while completions stay near-serial.
            all_loads = []
            g0, g1 = [], []
            for i in range(2 * NQ):
                g, q = divmod(i, NQ)
                sl, ld = load_slab(g, q, split=(i == 0 or i == NQ - 1))
                if i >= 2:
                    add_dep_helper(ld.ins, all_loads[i - 2].ins,
                                   reason="depth-2 load ladder")
                all_loads.append(ld)
                (g0 if g == 0 else g1).append((sl, ld))

            # group-0 pooling: 3 passes ScalarE + 1 DVE per slab, keeping
            # the DVE clear to start the blend right at kern0
            P0 = mlp.tile([128, U, NQ], FP32, tag="P")
            for q in range(NQ):
                if q == NQ - 1:
                    # last slab arrives in halves; finish its pools with a
                    # 2/2 engine split so kern0 isn't gated on one engine
                    pool_u(g0[q][0], q, P0, 0, on_act=True)
                    pool_u(g0[q][0], q, P0, 1, on_act=False)
                    pool_u(g0[q][0], q, P0, 2, on_act=True)
                    pool_u(g0[q][0], q, P0, 3, on_act=False)
                else:
                    pool_u(g0[q][0], q, P0, 0, on_act=True)
                    pool_u(g0[q][0], q, P0, 1, on_act=True)
                    pool_u(g0[q][0], q, P0, 2, on_act=True)
                    pool_u(g0[q][0], q, P0, 3, on_act=False)
            kern0 = gen_mlp(P0)

            # group-1 pooling rides ScalarE inside the group-0 blend window
            P1 = mlp.tile([128, U, NQ], FP32, tag="P")
            for q in range(NQ):
                for u in range(U):
                    pool_u(g1[q][0], q, P1, u, on_act=True)
            stores0 = []
            for q in range(NQ):
                # ScalarE also has room for half the k2 strips here
                stores0.append(blend(0, q, g0[q][0], kern0,
                                     b_on_act=(q % 2 == 1)))
            # stores yield HBM to the remaining loads
            last_ld = all_loads[-1]
            for st in stores0:
                add_dep_helper(st.ins, last_ld.ins,
                               reason="store yields HBM to loads")

            kern1 = gen_mlp(P1)
            # group-1 blend: ScalarE is free again, take every k2 strip
            for q in range(NQ):
                blend(1, q, g1[q][0], kern1, b_on_act=True)
    nc.finalize()
    return nc


def _get_nc():
    global _nc_cache
    if _nc_cache is None:
        _nc_cache = _build_nc()
    return _nc_cache


def _pack_small(W1, bn_gamma, bn_beta, bn_mean, bn_var, W2):
    W1 = np.asarray(W1, np.float32)
    W2 = np.asarray(W2, np.float32)
    gam = np.asarray(bn_gamma, np.float32)
    bet = np.asarray(bn_beta, np.float32)
    mea = np.asarray(bn_mean, np.float32)
    var = np.asarray(bn_var, np.float32)
    s = (gam / np.sqrt(var + np.float32(EPS))).astype(np.float32)
    t = (bet - mea * s).astype(np.float32)
    w1s = (W1 * s[:, None] * np.float32(1.0 / THW)).astype(np.float32)
    return np.concatenate(
        [w1s.reshape(-1), W2.reshape(-1), t]
    ).astype(np.float32)


def _ensure_hook_stub():
    """bass_utils' trace path imports antenv.axon_hooks when BASS_TRACE is
    set; if this image lacks it, register a None-returning stub so the run
    degrades to no-trace instead of crashing."""
    import sys
    import types

    try:
        import antenv.axon_hooks  # noqa: F401
    except ImportError:
        mod = types.ModuleType("antenv.axon_hooks")
        mod.get_axon_ntff_profile_hook = lambda: None
        mod.set_axon_ntff_profile_hook = lambda h: None
        sys.modules["antenv.axon_hooks"] = mod


def kernel(x, W1, bn_gamma, bn_beta, bn_mean, bn_var, W2):
    global last_results
    _ensure_hook_stub()
    nc = _get_nc()
    x = np.ascontiguousarray(np.asarray(x, dtype=np.float32)).reshape(
        N_CORES, U, C, THW
    ).astype(ml_dtypes.bfloat16)
    wpack = _pack_small(W1, bn_gamma, bn_beta, bn_mean, bn_var, W2)
    in_maps = [{"x": x[i], "wpack": wpack} for i in range(N_CORES)]
    last_results = run_bass_kernel_spmd(nc, in_maps, list(range(N_CORES)))
    out = np.stack([last_results.results[i]["out"] for i in range(N_CORES)])
    return out.astype(np.float32).reshape(N_CORES * U, C, T, H, W)


# revision 24
# speedup vs baseline: 1.0761x; 1.0761x over previous
"""Trainium2 Bass kernel for the dynamic segment-aggregation module.

Computation per (clip n, channel c):
  pooled[u]  = mean_{t,h,w} x[n,c,u,...]                (U=4 segments)
  z          = relu(pooled @ W1'^T + t_row)             (BN folded into W1/t)
  kern       = softmax(z @ W2^T)                        (K=3 taps)
  out[u]     = kern[0]*x[u-1] + kern[1]*x[u] + kern[2]*x[u+1]   (zero-padded)

Sharding: data-parallel over the 8 clips -> 1 clip (4 U-segments) per
NeuronCore; tiny generator weights replicated (one 64-float tensor with
BN affine and the 1/THW pooling mean folded in host-side).

The big lever vs the fp32 version: x and out travel through HBM as
bf16 (host casts both ways), halving DMA traffic to ~25.7 MB/core
(~72 us at 358 GB/s) and unlocking the DVE 2x (2-tensor) / 4x
(1-tensor) perf modes.  The blend is fused into one ScalarE tap-strip
(k2*x[1:4], FD=4704) plus four DVE ops per slab:
    out[0]   = k1*x[0]   + t[0]          (STT, FD=1568)
    p[0:2]   = k1*x[1:3] + t[1:3]        (STT, FD=3136)
    p[2]     = k1*x[3]                   (TS,  FD=1568, 4x)
    out[1:4] = k0*x[0:3] + p[0:3]        (STT, FD=4704)
Group-0 pooling rides DVE during the load phase; group-1 pooling is
split ScalarE/DVE inside the group-0 blend window.  All 8 slab loads
are queued up-front on the Sync HWDGE ring; group-0 stores defer
behind the last load so loads keep HBM priority.
"""

import numpy as np
import ml_dtypes

import concourse.bass as bass
import concourse.bacc as bacc
import concourse.tile as tile
from concourse import mybir
from concourse.bass_utils import run_bass_kernel_spmd

U = 4          # segments per clip
C = 256        # channels
T, H, W = 8, 28, 28
THW = T * H * W            # 6272
NQ = 4                     # t-quarters per channel-group
FQ = THW // NQ             # 1568
D = 8                      # MLP hidden dim (U * alpha)
K = 3                      # conv taps
EPS = 1e-5
N_CORES = 8
NCG = C // 128             # channel groups per core

# packed small-weights layout: [W1*s/THW (D*U) | W2 (K*D) | t_row (D)]
NPACK = D * U + K * D + D    # 64

FP32 = mybir.dt.float32
BF16 = mybir.dt.bfloat16

_nc_cache = None
last_results = None        # BassKernelResults of the most recent run (for test.py)


def _bcast_ap(ap, parts=128):
    """DRAM AP replicated across `parts` partitions (partition stride 0)."""
    return bass.AP(tensor=ap.tensor, offset=ap.offset, ap=[[0, parts]] + list(ap.ap))


def _bc_free(ap, n):
    """Insert a stride-0 broadcast dim of length n right after the
    partition dim of an SBUF AP."""
    dims = [list(d) for d in ap.ap]
    return bass.AP(
        tensor=ap.tensor, offset=ap.offset, ap=[dims[0]] + [[0, n]] + dims[1:]
    )


def _build_nc():
    nc = bacc.Bacc(None, target_bir_lowering=False)
    x_h = nc.declare_dram_parameter("x", [U, C, THW], BF16, isOutput=False)
    wp_h = nc.declare_dram_parameter("wpack", [NPACK], FP32, isOutput=False)
    out_h = nc.declare_dram_parameter("out", [U, C, THW], BF16, isOutput=True)

    xg = x_h[:].rearrange("u c f -> c u f")      # [C, U, THW]
    og = out_h[:].rearrange("u c f -> c u f")

    AX = mybir.AxisListType
    OP = mybir.AluOpType
    AF = mybir.ActivationFunctionType

    with tile.TileContext(nc) as tc:
        with (
            tc.tile_pool(name="xp", bufs=8) as xp,
            tc.tile_pool(name="outp", bufs=4) as outp,
            tc.tile_pool(name="tp", bufs=2) as tp,
            tc.tile_pool(name="ppp", bufs=2) as ppp,
            tc.tile_pool(name="small", bufs=1) as small,
            tc.tile_pool(name="mlp", bufs=2) as mlp,
        ):
            # one tiny DMA for every per-core-replicated constant
            wpk = small.tile([128, NPACK], FP32)
            nc.gpsimd.dma_start(out=wpk, in_=_bcast_ap(wp_h[:]))
            w1sb = wpk[:, 0:D * U].rearrange("p (d u) -> p d u", d=D)       # [128,D,U]
            w2sb = wpk[:, D * U:D * U + K * D].rearrange(
                "p (k d) -> p k d", k=K)                                    # [128,K,D]
            t_row = wpk[:, D * U + K * D:NPACK]                             # [128,D]

            def load_slab(g, q, split=False):
                c0 = g * 128
                sl = xp.tile([128, U, FQ], BF16, tag="xslab")
                if split:
                    # two half-loads so the first pools can start sooner
                    nc.sync.dma_start(
                        out=sl[:, 0:2, :],
                        in_=xg[c0:c0 + 128, 0:2, q * FQ:(q + 1) * FQ],
                    )
                    ld = nc.sync.dma_start(
                        out=sl[:, 2:4, :],
                        in_=xg[c0:c0 + 128, 2:4, q * FQ:(q + 1) * FQ],
                    )
                else:
                    ld = nc.sync.dma_start(
                        out=sl, in_=xg[c0:c0 + 128, :, q * FQ:(q + 1) * FQ]
                    )
                return sl, ld

            def pool_u(sl, q, P, u, on_act):
                """One per-u pooling pass.  ScalarE: activation accumulator
                (self-copy carrier).  DVE: tensor_reduce (accum_out on DVE
                drops to 1x anyway; reduce avoids the dummy write)."""
                if on_act:
                    nc.scalar.activation(
                        out=sl[:, u, :], in_=sl[:, u, :], func=AF.Copy,
                        accum_out=P[:, u, q:q + 1],
                    )
                else:
                    nc.vector.reduce_sum(
                        out=P[:, u, q:q + 1], in_=sl[:, u, :], axis=AX.X
                    )

            def gen_mlp(P):
                """pooled -> relu(pooled@W1'^T + t_row) -> softmax(z@W2^T)."""
                pooled = mlp.tile([128, U], FP32, tag="pooled")
                nc.vector.reduce_sum(out=pooled, in_=P, axis=AX.X)
                zw = mlp.tile([128, D, U], FP32, tag="zw")
                nc.vector.tensor_mul(
                    out=zw, in0=w1sb, in1=_bc_free(pooled[:, :], D)
                )
                z = mlp.tile([128, D], FP32, tag="z")
                nc.vector.reduce_sum(out=z, in_=zw, axis=AX.X)
                nc.vector.tensor_add(out=z, in0=z, in1=t_row)
                nc.vector.tensor_scalar_max(out=z, in0=z, scalar1=0.0)
                lw = mlp.tile([128, K, D], FP32, tag="lw")
                nc.vector.tensor_mul(
                    out=lw, in0=w2sb, in1=_bc_free(z[:, :], K)
                )
                logit = mlp.tile([128, K], FP32, tag="logit")
                nc.vector.reduce_sum(out=logit, in_=lw, axis=AX.X)
                mx = mlp.tile([128, 1], FP32, tag="mx")
                nc.vector.reduce_max(out=mx, in_=logit, axis=AX.X,
                                     negate=True)
                nc.scalar.activation(
                    out=logit, in_=logit, func=AF.Exp, bias=mx[:, 0:1]
                )
                ssum = mlp.tile([128, 1], FP32, tag="ssum")
                nc.vector.reduce_sum(out=ssum, in_=logit, axis=AX.X)
                nc.vector.reciprocal(out=ssum, in_=ssum)
                kern = mlp.tile([128, K], FP32, tag="kern")
                nc.vector.tensor_scalar_mul(out=kern, in0=logit, scalar1=ssum[:, 0:1])
                return kern

            def make_b(sl, kern, on_act):
                """B = k2 * sl[1:4] as its own op so its engine/FIFO slot
                can be scheduled independently of the rest of the blend."""
                B = tp.tile([128, 3, FQ], BF16, tag="B")
                if on_act:
                    nc.scalar.activation(
                        out=B, in_=sl[:, 1:4, :], func=AF.Copy,
                        scale=kern[:, 2:3]
                    )
                else:
                    nc.vector.tensor_scalar_mul(
                        out=B, in0=sl[:, 1:4, :], scalar1=kern[:, 2:3]
                    )
                return B

            def blend(g, q, sl, kern, b_on_act, B=None):
                """One slab blend.  STT is 1x on this silicon, so the MAC
                stream is TS products (4x) + in-place TT adds (2x):
                  ot      = k1*sl          (TS, FD 6272)
                  B       = k2*sl[1:4]     (TS, FD 4704; ScalarE when it
                                            has slack -- b_on_act)
                  Cc      = k0*sl[0:3]     (TS, FD 4704)
                  ot[0:3] += B             (TT, FD 4704)
                  ot[1:3] += Cc[0:2]       (TT, FD 3136)
                  ot[3]   += Cc[2]         (TT on GpSimd, FD 1568)
                Returns the store handle."""
                c0 = g * 128
                k0, k1, k2 = kern[:, 0:1], kern[:, 1:2], kern[:, 2:3]
                ot = outp.tile([128, U, FQ], BF16, tag="outslab")
                nc.vector.tensor_scalar_mul(out=ot, in0=sl, scalar1=k1)
                if B is None:
                    B = make_b(sl, kern, b_on_act)
                Cc = ppp.tile([128, 3, FQ], BF16, tag="Cc")
                nc.vector.tensor_scalar_mul(
                    out=Cc, in0=sl[:, 0:3, :], scalar1=k0
                )
                # two in-place TT adds; both FD 4704 at 2x.  (No GpSimd
                # here: concurrent Q7 SBUF traffic knocks the DVE 4x port
                # mode down to 1x — observed 0.95 -> 3.85 us.)
                nc.vector.tensor_add(
                    out=ot[:, 0:3, :], in0=ot[:, 0:3, :], in1=B
                )
                nc.vector.tensor_add(
                    out=ot[:, 1:4, :], in0=ot[:, 1:4, :], in1=Cc
                )
                st = nc.gpsimd.dma_start(
                    out=og[c0:c0 + 128, :, q * FQ:(q + 1) * FQ], in_=ot
                )
                return st

            def blend_tail(g, q, sl, kern):
                """Last-slab blend in two u-pair chunks so the first
                half-store departs while the second half computes."""
                c0 = g * 128
                k0, k1, k2 = kern[:, 0:1], kern[:, 1:2], kern[:, 2:3]
                ot = outp.tile([128, U, FQ], BF16, tag="outslab")
                # chunk A: u0 = k1*x0 + k2*x1 ; u1 = k1*x1 + k2*x2 + k0*x0
                nc.vector.tensor_scalar_mul(
                    out=ot[:, 0:2, :], in0=sl[:, 0:2, :], scalar1=k1
                )
                Bt = tp.tile([128, 3, FQ], BF16, tag="B")
                nc.scalar.activation(
                    out=Bt[:, 0:2, :], in_=sl[:, 1:3, :], func=AF.Copy,
                    scale=k2
                )
                nc.vector.tensor_add(
                    out=ot[:, 0:2, :], in0=ot[:, 0:2, :], in1=Bt[:, 0:2, :]
                )
                Ct = ppp.tile([128, 3, FQ], BF16, tag="Cc")
                nc.vector.tensor_scalar_mul(
                    out=Ct[:, 0:1, :], in0=sl[:, 0:1, :], scalar1=k0
                )
                nc.vector.tensor_add(
                    out=ot[:, 1:2, :], in0=ot[:, 1:2, :], in1=Ct[:, 0:1, :]
                )
                nc.gpsimd.dma_start(
                    out=og[c0:c0 + 128, 0:2, q * FQ:(q + 1) * FQ],
                    in_=ot[:, 0:2, :],
                )
                # chunk B: u2 = k1*x2 + k2*x3 + k0*x1 ; u3 = k1*x3 + k0*x2
                nc.vector.tensor_scalar_mul(
                    out=ot[:, 2:4, :], in0=sl[:, 2:4, :], scalar1=k1
                )
                nc.scalar.activation(
                    out=Bt[:, 2:3, :], in_=sl[:, 3:4, :], func=AF.Copy,
                    scale=k2
                )
                nc.vector.tensor_add(
                    out=ot[:, 2:3, :], in0=ot[:, 2:3, :], in1=Bt[:, 2:3, :]
                )
                nc.vector.tensor_scalar_mul(
                    out=Ct[:, 1:3, :], in0=sl[:, 1:3, :], scalar1=k0
                )
                nc.vector.tensor_add(
                    out=ot[:, 2:4, :], in0=ot[:, 2:4, :], in1=Ct[:, 1:3, :]
                )
                nc.gpsimd.dma_start(
                    out=og[c0:c0 + 128, 2:4, q * FQ:(q + 1) * FQ],
                    in_=ot[:, 2:4, :],
                )

            # ---- schedule ----
            from concourse.tile_rust import add_dep_helper

            # Loads laddered two-deep: with all 8 queued at once the HWDGE
            # ring interleaves them and the FIRST slab only lands at ~17us
            # (observed), stalling pooling.  Depth-2 keeps the pipe full
            # while completions stay near-serial.
            all_loads = []
            g0, g1 = [], []
            for i in range(2 * NQ):
                g, q = divmod(i, NQ)
                sl, ld = load_slab(g, q, split=(i == 0 or i == NQ - 1))
                if i >= 2:
                    add_dep_helper(ld.ins, all_loads[i - 2].ins,
                                   reason="depth-2 load ladder")
                all_loads.append(ld)
                (g0 if g == 0 else g1).append((sl, ld))

            # group-0 pooling: 2 ScalarE + 2 DVE per slab (ScalarE alone
            # can't keep up with the 4.5 us load cadence at 3/slab)
            P0 = mlp.tile([128, U, NQ], FP32, tag="P")
            for q in range(NQ):
                pool_u(g0[q][0], q, P0, 0, on_act=True)
                pool_u(g0[q][0], q, P0, 1, on_act=False)
                pool_u(g0[q][0], q, P0, 2, on_act=True)
                pool_u(g0[q][0], q, P0, 3, on_act=False)
            kern0 = gen_mlp(P0)

            # ScalarE's share of the group-0 k2 strips goes into its FIFO
            # BEFORE the group-1 pools so it can't starve the blend adds
            b0 = {}
            for q in (1, 3):
                b0[q] = make_b(g0[q][0], kern0, on_act=True)

            # group-1 pooling rides ScalarE inside the group-0 blend window
            P1 = mlp.tile([128, U, NQ], FP32, tag="P")
            for q in range(NQ):
                for u in range(U):
                    pool_u(g1[q][0], q, P1, u, on_act=True)
            stores0 = []
            for q in range(NQ):
                stores0.append(blend(0, q, g0[q][0], kern0,
                                     b_on_act=False, B=b0.get(q)))
            # stores yield HBM to the remaining loads
            last_ld = all_loads[-1]
            for st in stores0:
                add_dep_helper(st.ins, last_ld.ins,
                               reason="store yields HBM to loads")

            kern1 = gen_mlp(P1)
            # group-1 blend: ScalarE is free again, take the k2 strips
            # (q0's on DVE: right at kern1 ScalarE still has Exp latency).
            # Last slab is chunked into u-pairs so its first half-store
            # leaves ~4 us before the full blend would finish.
            for q in range(NQ - 1):
                blend(1, q, g1[q][0], kern1, b_on_act=(q != 0))
            blend_tail(1, NQ - 1, g1[NQ - 1][0], kern1)
    nc.finalize()
    return nc


def _get_nc():
    global _nc_cache
    if _nc_cache is None:
        _nc_cache = _build_nc()
    return _nc_cache


def _pack_small(W1, bn_gamma, bn_beta, bn_mean, bn_var, W2):
    W1 = np.asarray(W1, np.float32)
    W2 = np.asarray(W2, np.float32)
    gam = np.asarray(bn_gamma, np.float32)
    bet = np.asarray(bn_beta, np.float32)
    mea = np.asarray(bn_mean, np.float32)
    var = np.asarray(bn_var, np.float32)
    s = (gam / np.sqrt(var + np.float32(EPS))).astype(np.float32)
    t = (bet - mea * s).astype(np.float32)
    w1s = (W1 * s[:, None] * np.float32(1.0 / THW)).astype(np.float32)
    return np.concatenate(
        [w1s.reshape(-1), W2.reshape(-1), t]
    ).astype(np.float32)


def _ensure_hook_stub():
    """bass_utils' trace path imports antenv.axon_hooks when BASS_TRACE is
    set; if this image lacks it, register a None-returning stub so the run
    degrades to no-trace instead of crashing."""
    import sys
    import types

    try:
        import antenv.axon_hooks  # noqa: F401
    except ImportError:
        mod = types.ModuleType("antenv.axon_hooks")
        mod.get_axon_ntff_profile_hook = lambda: None
        mod.set_axon_ntff_profile_hook = lambda h: None
        sys.modules["antenv.axon_hooks"] = mod


def kernel(x, W1, bn_gamma, bn_beta, bn_mean, bn_var, W2):
    global last_results
    _ensure_hook_stub()
    nc = _get_nc()
    x = np.ascontiguousarray(np.asarray(x, dtype=np.float32)).reshape(
        N_CORES, U, C, THW
    ).astype(ml_dtypes.bfloat16)
    wpack = _pack_small(W1, bn_gamma, bn_beta, bn_mean, bn_var, W2)
    in_maps = [{"x": x[i], "wpack": wpack} for i in range(N_CORES)]
    last_results = run_bass_kernel_spmd(nc, in_maps, list(range(N_CORES)))
    out = np.stack([last_results.results[i]["out"] for i in range(N_CORES)])
    return out.astype(np.float32).reshape(N_CORES * U, C, T, H, W)
